# revision 50
# baseline (speedup 1.0000x reference)
"""Bass/Tile TRN2 kernel for multi-head self-attention with relative position bias.

Problem: B=4, T=2048, DIM=1024, HEADS=16, DH=64, causal + rel-pos-bias softmax.

Sharding (8 cores): data-parallel over batch (4) x tensor-parallel over heads (2x8).
Each core computes, for its (batch, 8-head group):
    qkv projection -> per-head causal attention (scoresT layout) -> partial out proj.
Host sums the two head-group partials per batch.

Layout choices (all matmuls in fp16):
 - x is pre-transposed on host: xT [DIM, T] so q/k land as [dh, t] ("T layouts")
   and v as [t, dh] with zero on-device transposes.
 - scores are computed transposed (scoresT[j, i] = k_j . q_i) so that
   exp(scoresT) can be used directly as the moving operand of the AV matmul and
   softmax sums come free via a ones-column appended to v.
 - rel-pos bias + causal mask are precomputed per head on host as a shifted
   multiplicative matrix Bh[p, c] = exp(bias(idx) | -inf), idx = p - c + 2431,
   so every scoresT tile's bias is a strided slice, applied post-exp on DVE.
 - the head pair (2g, 2g+1) lives at partition rows 0:64 / 64:128 of qT/kT, so
   their dh=64-contraction scores matmuls occupy disjoint PE row groups and
   run concurrently when issued back-to-back.
 - scores psum tiles hold 2 j-blocks ([128, 2, 512] = 2 banks) so exp and the
   bias multiply run as single 1024-wide ACT/DVE ops.
 - softmax normalization is deferred: the AV accumulator drains to SBUF at
   once (fast PSUM recycle); the sums row then bounces through DRAM for a
   partition-broadcast, is reciprocal'd and multiplied in on DVE one
   head-pair block later.
 - stage 1 (qkv) is chunk-interleaved with attention: attention on i-chunk ic
   overlaps the qkv projection of chunk ic+1 and the out-projection of chunk
   ic-1, keeping PE busy while ACT does exps.
"""

import os

import numpy as np

import concourse.bass as bass
import concourse.tile as tile
from concourse import bacc, mybir
from concourse.bass_utils import run_bass_kernel_spmd

B, T, DIM, HEADS, DH = 4, 2048, 1024, 16, 64
N_CORES = 8
HPC = HEADS // 2          # heads per core = 8
FQ = HPC * DH             # per-core q/k/v feature width = 512
BH_C = 2432               # bias matrix free size  (max shift 1920 + 512)

F32 = mybir.dt.float32
F16 = mybir.dt.float16
DTM = F16                 # matmul-input dtype
EXP = mybir.ActivationFunctionType.Exp
COPY = mybir.ActivationFunctionType.Copy
MULT = mybir.AluOpType.mult

_CACHE = {}


def build_nc():
    nc = bacc.Bacc("TRN2", target_bir_lowering=False, debug=False,
                   enable_asserts=True, num_devices=N_CORES)
    xT_d = nc.dram_tensor("xT", [DIM, T], DTM, kind="ExternalInput").ap()
    wq_d = nc.dram_tensor("wq", [DIM, FQ], DTM, kind="ExternalInput").ap()
    wk_d = nc.dram_tensor("wk", [DIM, FQ], DTM, kind="ExternalInput").ap()
    wv_d = nc.dram_tensor("wv", [DIM, FQ], DTM, kind="ExternalInput").ap()
    w0_d = nc.dram_tensor("w0", [FQ, DIM], DTM, kind="ExternalInput").ap()
    bh_d = nc.dram_tensor("bh", [HPC, 128, BH_C], DTM, kind="ExternalInput").ap()
    out_d = nc.dram_tensor("out", [T, DIM], F32, kind="ExternalOutput").ap()
    kdebug = os.environ.get("KDEBUG")
    if kdebug:
        ao_dbg = nc.dram_tensor("ao_dbg", [128, 4, T], DTM,
                                kind="ExternalOutput").ap()
        e_dbg = nc.dram_tensor("e_dbg", [128, 2, 512], DTM,
                               kind="ExternalOutput").ap()
        rb_dbg = nc.dram_tensor("rb_dbg", [64, 512], F32,
                                kind="ExternalOutput").ap()
        sc_dbg = nc.dram_tensor("sc_dbg", [128, 2, 512], DTM,
                                kind="ExternalOutput").ap()

    krepeat = int(os.environ.get("KREPEAT", "1"))
    ablate = os.environ.get("KABLATE", "")
    with tile.TileContext(nc) as tc:
      import contextlib
      loop_cm = (tc.For_i(0, krepeat, 1, staggered_reset=True,
                            hint_engines=(mybir.EngineType.PE, mybir.EngineType.Activation, mybir.EngineType.DVE))
                 if krepeat > 1 else contextlib.nullcontext())
      with tc.tile_pool(name="cst", bufs=1) as cst, \
           tc.tile_pool(name="persist", bufs=1) as persist:
        # ---- persistent SBUF state (outside the bench loop) ----
        qT = persist.tile([128, 4, T], DTM)       # q^T: [(h%2)*64+d, g, t]
        kT = persist.tile([128, 4, T], DTM)
        v_sb = persist.tile([128, 16, HPC, DH + 1], DTM)  # [t%128, t//128, h, d|1]
        aoT = persist.tile([128, 4, T], DTM)      # attn-out^T rows (h%2)*64+d
        wq_sb = persist.tile([128, 8, FQ], DTM)
        wk_sb = persist.tile([128, 8, FQ], DTM)
        wv_sb = persist.tile([128, 8, FQ], DTM)
        w0_sb = persist.tile([128, 4, DIM], DTM)
        bh_sb = persist.tile([128, HPC, BH_C], DTM)

        def load_invariants():
            """Weight/bias loads. In bench (KREPEAT) mode these are emitted
            once as a prologue and again at each body END, so the next
            iteration's weights prefetch behind the current tail instead of
            gating the first matmuls after the back-edge."""
            for kd in range(8):
                nc.scalar.dma_start(wk_sb[:, kd, :], wk_d[kd * 128:(kd + 1) * 128, :])
            for kd in range(8):
                nc.scalar.dma_start(wq_sb[:, kd, :], wq_d[kd * 128:(kd + 1) * 128, :])
            for kd in range(8):
                nc.scalar.dma_start(wv_sb[:, kd, :], wv_d[kd * 128:(kd + 1) * 128, :])
            for kf in range(4):
                nc.sync.dma_start(w0_sb[:, kf, :], w0_d[kf * 128:(kf + 1) * 128, :])
            for h in range(HPC):
                nc.scalar.dma_start(bh_sb[:, h, :], bh_d[h])

        with loop_cm:
            _es = contextlib.ExitStack()
            psS = _es.enter_context(tc.tile_pool(name="psS", bufs=3, space="PSUM"))
            psP = _es.enter_context(tc.tile_pool(name="psP", bufs=1, space="PSUM"))
            ps1 = _es.enter_context(tc.tile_pool(name="ps1", bufs=1, space="PSUM"))
            xpool = _es.enter_context(tc.tile_pool(name="xpool", bufs=16))
            ep = _es.enter_context(tc.tile_pool(name="ep", bufs=8))
            npool = _es.enter_context(tc.tile_pool(name="npool", bufs=4))
            dramp = _es.enter_context(tc.tile_pool(name="dramp", bufs=4, space="DRAM"))
            stgp = _es.enter_context(tc.tile_pool(name="stgp", bufs=3))
            osp = _es.enter_context(tc.tile_pool(name="osp", bufs=4))

            load_invariants()
            nc.vector.memset(v_sb[:, :, :, DH], 1.0)
            # prewarm the ACT exp table set (~2.7us) while stage 1 runs on PE
            warm = cst.tile([1, 2], F32)
            nc.vector.memset(warm, 0.0)
            nc.scalar.activation(warm, warm, EXP)

            def load_x_chunk(tci):
                t0 = tci * 512
                xts = []
                for kd in range(8):
                    xt1 = xpool.tile([128, 512], DTM, name=f"xt{tci}_{kd}",
                                     tag="xt")
                    nc.sync.dma_start(
                        xt1, xT_d[kd * 128:(kd + 1) * 128, t0:t0 + 512])
                    xts.append(xt1)
                return xts

            def qkv_piece(tci, xts, piece, wide=False):
                """One of 12 psum-tile units for chunk tci.
                piece 0-7: q/k (mb), piece 8-11: v (tt). During chunk 0 the
                scores pool is idle, so `wide` borrows its 2-bank slots for
                extra buffering."""
                t0 = tci * 512
                if wide:
                    ps = psS.tile([128, 2, 512], F32, name="sc", tag="sc")[:, 0, :]
                else:
                    ps = ps1.tile([128, 512], F32, name="ps1", tag="mm1")
                if piece < 8:
                    mb = piece
                    w_sb, fb = (wq_sb, mb) if mb < 4 else (wk_sb, mb - 4)
                    for kd in range(8):
                        nc.tensor.matmul(
                            ps, w_sb[:, kd, fb * 128:(fb + 1) * 128],
                            xts[kd], start=(kd == 0), stop=(kd == 7))
                    dst = qT if mb < 4 else kT
                    nc.vector.tensor_copy(dst[:, fb, t0:t0 + 512], ps)
                else:
                    tt = piece - 8
                    for kd in range(8):
                        nc.tensor.matmul(
                            ps, xts[kd][:, tt * 128:(tt + 1) * 128],
                            wv_sb[:, kd, :], start=(kd == 0), stop=(kd == 7))
                    tb = tci * 4 + tt
                    nc.scalar.activation(
                        v_sb[:, tb, :, 0:DH],
                        ps.rearrange("p (h d) -> p h d", h=HPC), COPY)

            def attn_front(g, ic, n):
                """SC + exp + bias-mult for pair-group n of (g, ic); returns
                the two heads' e tiles. Scores of the two heads issue
                adjacently so their row-disjoint 64-contraction matmuls
                overlap on the PE. The last group of each (g, ic) is >=half
                causally masked (its j-blocks start 256/384 columns into the
                i-window), so all its work runs on the 256-column suffix."""
                i0 = ic * 512
                jb0, jb1 = 2 * n, 2 * n + 1
                tr = 256 if n == 2 * (ic + 1) - 1 else 0
                # psum layout dim1: k=0 <-> jb1, k=1 <-> jb0 (so the bias
                # window AP walks +128 columns from the jb1 offset)
                pss = []
                for idx in range(2):
                    hp = idx * 64
                    ps = psS.tile([128, 2, 512], F32, name="sc", tag="sc")
                    for k, jb in ((0, jb1), (1, jb0)):
                        nc.tensor.matmul(
                            ps[:, k, tr:512],
                            kT[hp:hp + 64, g, jb * 128:jb * 128 + 128],
                            qT[hp:hp + 64, g, i0 + tr:i0 + 512],
                            start=True, stop=True)
                    pss.append(ps)
                es = []
                for idx in range(2):
                    h = 2 * g + idx
                    e = ep.tile([128, 2, 512], DTM, name="e", tag="e")
                    if ablate == "lean":
                        nc.vector.tensor_copy(e, pss[idx])
                        es.append(e)
                        continue
                    nc.scalar.activation(e[:, :, tr:512], pss[idx][:, :, tr:512],
                                         EXP)
                    if kdebug and (g, ic, n, idx) == (0, 0, 0, 0):
                        nc.sync.dma_start(sc_dbg, e)
                    off1 = i0 - jb1 * 128 + 384
                    w = bh_sb[:, h, off1 + tr:off1 + 512]
                    bwin = bass.AP(
                        tensor=w.tensor, offset=w.offset,
                        ap=[list(w.ap[0]), [128, 2], list(w.ap[1])])
                    nc.vector.tensor_tensor(e[:, :, tr:512], e[:, :, tr:512],
                                            bwin, MULT)
                    if kdebug and (g, ic, n, idx) == (0, 0, 0, 0):
                        nc.sync.dma_start(e_dbg, e)
                    es.append(e)
                return es

            def attn_av(g, ic, n, nlast, es, pos_pair):
                """AV accumulation for pair-group n — issued with a software
                lag behind its SC/exp/mult so the in-order PE stream never
                waits on a fresh exp+mult chain."""
                jb0, jb1 = 2 * n, 2 * n + 1
                tr = 256 if n == nlast else 0
                for idx in range(2):
                    h = 2 * g + idx
                    for k, jb in ((1, jb0), (0, jb1)):
                        nc.tensor.matmul(
                            pos_pair[idx][:, tr:512], v_sb[:, jb, h, :],
                            es[idx][:, k, tr:512],
                            start=(jb == 0), stop=(jb == 2 * nlast + 1),
                            skip_group_check=True)

            def pos_drain(pos_pair):
                """Copy the pair's AV accumulators to SBUF so the PSUM banks
                recycle fast; the softmax division happens later, off the
                accumulator-recycle loop."""
                outs = []
                for idx in range(2):
                    sa = npool.tile([DH + 1, 512], F32, name="sa", tag="sa",
                                    bufs=6)
                    nc.vector.tensor_copy(sa, pos_pair[idx])
                    outs.append(sa)
                return outs

            def normalize(g, ic, sas):
                i0 = ic * 512
                for idx in range(2):
                    sa = sas[idx]
                    if ablate == "lean":
                        if idx == 0:
                            nc.vector.tensor_copy(
                                aoT[0:64, g, i0:i0 + 512], sa[0:DH, :])
                        else:
                            stg = stgp.tile([64, 512], DTM, name="stg",
                                            tag="stg")
                            nc.vector.tensor_copy(stg, sa[0:DH, :])
                            nc.gpsimd.dma_start(
                                out=aoT[64:128, g, i0:i0 + 512], in_=stg)
                        continue
                    r_dram = dramp.tile([1, 512], F32, name="rd")
                    # scalar HWDGE: idle after the weight/bias prologue, so
                    # the latency-critical bounce isn't stuck behind x-chunk
                    # loads and output stores on the sync ring
                    nc.scalar.dma_start(r_dram, sa[DH:DH + 1, :])
                    rb = npool.tile([64, 512], F32, name="rb", tag="rb")
                    rb_src = bass.AP(tensor=r_dram.tensor, offset=r_dram.offset,
                                     ap=[[0, 64]] + list(r_dram.ap[1:]))
                    nc.gpsimd.dma_start(out=rb, in_=rb_src)
                    rr = npool.tile([64, 512], F32, name="rr", tag="rr")
                    nc.vector.reciprocal(rr, rb)
                    if kdebug and (g, ic, idx) == (0, 0, 0):
                        nc.sync.dma_start(rb_dbg, rr)
                    # multiplies on the idle GpSimd engine: keeps the
                    # bounce-dependent op out of the DVE FIFO, where it can
                    # head-of-line-block the bias-multiply stream
                    if idx == 0:
                        nc.gpsimd.tensor_tensor(
                            aoT[0:64, g, i0:i0 + 512], sa[0:DH, :], rr, MULT)
                    else:
                        stg = stgp.tile([64, 512], DTM, name="stg", tag="stg")
                        nc.gpsimd.tensor_tensor(stg, sa[0:DH, :], rr, MULT)
                        nc.gpsimd.dma_start(
                            out=aoT[64:128, g, i0:i0 + 512], in_=stg)

            def stage3_piece(tb, ncol, wide=False):
                n0 = ncol * 512
                if wide:
                    ps = psS.tile([128, 2, 512], F32, name="sc",
                                  tag="sc")[:, 0, :]
                else:
                    ps = ps1.tile([128, 512], F32, name="ps1", tag="mm1")
                for kf in range(4):
                    nc.tensor.matmul(
                        ps, aoT[:, kf, tb * 128:(tb + 1) * 128],
                        w0_sb[:, kf, n0:n0 + 512],
                        start=(kf == 0), stop=(kf == 3))
                o_sb = osp.tile([128, 512], F32, name="osb", tag="osb")
                if (tb + ncol) % 2 == 0:
                    nc.scalar.activation(o_sb, ps, COPY)
                    nc.sync.dma_start(
                        out_d[tb * 128:(tb + 1) * 128, n0:n0 + 512], o_sb)
                else:
                    nc.vector.tensor_copy(o_sb, ps)
                    nc.scalar.dma_start(
                        out_d[tb * 128:(tb + 1) * 128, n0:n0 + 512], o_sb)

            # ---- pipeline ----
            if ablate == "dma":
                # loads + output stores only: measures the DMA floor
                for tci in range(4):
                    load_x_chunk(tci)
                o_sb = osp.tile([128, 512], F32, name="osb", tag="osb")
                nc.vector.memset(o_sb, 0.0)
                for tb in range(16):
                    for ncol in range(2):
                        nc.sync.dma_start(
                            out_d[tb * 128:(tb + 1) * 128,
                                  ncol * 512:ncol * 512 + 512], o_sb)
            elif ablate == "noattn":
                # stage 1 + stage 3 only (aoT = zeros)
                nc.vector.memset(aoT, 0.0)
                for tci in range(4):
                    xts = load_x_chunk(tci)
                    for piece in range(12):
                        qkv_piece(tci, xts, piece)
                for tb in range(16):
                    for ncol in range(2):
                        stage3_piece(tb, ncol)
            elif ablate in ("dve2d", "dve3d"):
                # DVE micro-bench: 400 fp16 [128,1024] multiplies,
                # contiguous (2d) or bias-window-strided in2 (3d)
                ea = ep.tile([128, 2, 512], DTM, name="e", tag="e")
                nc.vector.memset(ea, 1.0)
                nc.vector.memset(bh_sb[:, 0, :], 1.0)
                for it in range(400):
                    off1 = (it * 128) % 1280 + 128
                    if ablate == "dve2d":
                        bw = bh_sb[:, 0, off1:off1 + 1024]
                        bw = bass.AP(tensor=bw.tensor, offset=bw.offset,
                                     ap=[list(bw.ap[0]), [512, 2], [1, 512]])
                    else:
                        w = bh_sb[:, 0, off1:off1 + 512]
                        bw = bass.AP(tensor=w.tensor, offset=w.offset,
                                     ap=[list(w.ap[0]), [128, 2], [1, 512]])
                    nc.vector.tensor_tensor(ea, ea, bw, MULT)
                nc.sync.dma_start(out_d[0:128, 0:512].bitcast(DTM),
                                  ea.rearrange("p a b -> p (a b)"))
            elif ablate == "attnonly":
                # attention only on memset activations (no stage 1/3)
                nc.vector.memset(qT, 0.01)
                nc.vector.memset(kT, 0.01)
                nc.vector.memset(v_sb[:, :, :, 0:DH], 0.01)
                deferred = []
                for ic in range(4):
                    ngrp = 2 * (ic + 1)
                    for g in range(4):
                        pos_pair = [
                            psP.tile([DH + 1, 512], F32, name=f"po{idx}",
                                     tag="po")
                            for idx in range(2)]
                        pend = []
                        for n in range(ngrp):
                            pend.append((n, attn_front(g, ic, n)))
                            if len(pend) > 2:
                                m, es = pend.pop(0)
                                attn_av(g, ic, m, ngrp - 1, es, pos_pair)
                            if deferred:
                                normalize(*deferred.pop(0))
                        for m, es in pend:
                            attn_av(g, ic, m, ngrp - 1, es, pos_pair)
                        deferred.append((g, ic, pos_drain(pos_pair)))
                for item in deferred:
                    normalize(*item)
                nc.sync.dma_start(out_d[0:128, 0:512].bitcast(DTM),
                                  aoT[:, 0, 0:1024])
            else:
                # chunk 0: k/q(g=0)/v first so attention(0) can start earliest;
                # alternate between the idle scores-pool slots and ps1.
                xts = load_x_chunk(0)
                for i, piece in enumerate([4, 5, 6, 7, 0, 8, 9, 10, 11, 1, 2, 3]):
                    qkv_piece(0, xts, piece, wide=(i % 3 != 2))

                stage3_todo = []
                deferred = []
                for ic in range(4):
                    if ic < 3:
                        xts_next = load_x_chunk(ic + 1)
                    ngrp = 2 * (ic + 1)
                    piece = 0
                    for g in range(4):
                        pos_pair = [
                            psP.tile([DH + 1, 512], F32, name=f"po{idx}",
                                     tag="po")
                            for idx in range(2)]
                        pend = []
                        for n in range(ngrp):
                            pend.append((n, attn_front(g, ic, n)))
                            if len(pend) > 2:
                                m, es = pend.pop(0)
                                attn_av(g, ic, m, ngrp - 1, es, pos_pair)
                            # interleave stage-1 pieces of the next chunk
                            if ic < 3 and n % 2 == 1 and piece < 12:
                                qkv_piece(ic + 1, xts_next, piece)
                                piece += 1
                            # interleave deferred out-projection tiles
                            elif stage3_todo and n % 2 == 1:
                                stage3_piece(*stage3_todo.pop(0))
                            # finish a deferred softmax division last so its
                            # bounce DMA has maximal slack before the DVE
                            # stream reaches the dependent multiply
                            if deferred:
                                normalize(*deferred.pop(0))
                        for m, es in pend:
                            attn_av(g, ic, m, ngrp - 1, es, pos_pair)
                        deferred.append((g, ic, pos_drain(pos_pair)))
                    if ic < 3:
                        while piece < 12:
                            qkv_piece(ic + 1, xts_next, piece)
                            piece += 1
                    while deferred:
                        normalize(*deferred.pop(0))
                    for tb in range(4 * ic, 4 * ic + 4):
                        for ncol in range(2):
                            stage3_todo.append((tb, ncol))
                for i, (tb, ncol) in enumerate(stage3_todo):
                    stage3_piece(tb, ncol, wide=(i % 3 != 2))
                if kdebug:
                    nc.sync.dma_start(ao_dbg, aoT)
            _es.close()
    nc.compile()
    return nc


def conv(a):
    return np.ascontiguousarray(a, dtype=np.float32).astype(np.float16)


def prep_inputs(x, W_qkv, W_0, rel_bias):
    """Shard + lay out the full inputs into 8 per-core input maps."""
    x = np.asarray(x, dtype=np.float32)
    W_qkv = np.asarray(W_qkv, dtype=np.float32)
    W_0 = np.asarray(W_0, dtype=np.float32)
    rel_bias = np.asarray(rel_bias, dtype=np.float32)

    # W_qkv columns are laid out (d, s, h): col = d*48 + s*16 + h
    wslab = W_qkv.reshape(DIM, DH, 3, HEADS)

    # bias matrices: bh_all[h, p, c] = exp(bias) | 0 at idx = p - c + 2431
    p = np.arange(128)[:, None]
    c = np.arange(BH_C)[None, :]
    idx = p - c + 2431                       # [128, C]
    safe = np.clip(idx, 0, 2 * T - 2)
    base = rel_bias[safe, :]                 # [128, C, HEADS]
    invalid = (idx < 0) | (idx > 2 * T - 2)
    masked = idx > T - 1                     # j > i  -> causal mask
    bh_all = np.where(masked[..., None], np.float32(-60000.0),
                      np.where(invalid[..., None], np.float32(0.0), base))
    bh_all = np.transpose(bh_all, (2, 0, 1)).copy()  # [HEADS, 128, C]
    bh_all = np.exp(bh_all)  # multiplicative form; exp(-60000) -> 0
    bh_all = conv(bh_all)

    in_maps = []
    for core in range(N_CORES):
        b, hg = divmod(core, 2)
        h0 = hg * HPC
        # per-core weight slices, feature order f = h*64 + d
        wq = wslab[:, :, 0, h0:h0 + HPC].transpose(0, 2, 1).reshape(DIM, FQ)
        wq = wq * np.float32(DH ** -0.5)
        wk = wslab[:, :, 1, h0:h0 + HPC].transpose(0, 2, 1).reshape(DIM, FQ)
        wv = wslab[:, :, 2, h0:h0 + HPC].transpose(0, 2, 1).reshape(DIM, FQ)
        in_maps.append({
            "xT": conv(x[b].T),
            "wq": conv(wq),
            "wk": conv(wk),
            "wv": conv(wv),
            "w0": conv(W_0[h0 * DH:(h0 + HPC) * DH, :]),
            "bh": np.ascontiguousarray(bh_all[h0:h0 + HPC]),
        })
    return in_maps


def kernel(x, W_qkv, W_0, rel_bias):
    if "nc" not in _CACHE:
        _CACHE["nc"] = build_nc()
    nc = _CACHE["nc"]
    in_maps = prep_inputs(x, W_qkv, W_0, rel_bias)
    res = run_bass_kernel_spmd(nc, in_maps, core_ids=list(range(N_CORES)))
    out = np.empty((B, T, DIM), dtype=np.float32)
    for b in range(B):
        out[b] = res.results[2 * b]["out"] + res.results[2 * b + 1]["out"]
    return out


# revision 51
# speedup vs baseline: 1.0444x; 1.0444x over previous
"""Bass/Tile TRN2 kernel for multi-head self-attention with relative position bias.

Problem: B=4, T=2048, DIM=1024, HEADS=16, DH=64, causal + rel-pos-bias softmax.

Sharding (8 cores): data-parallel over batch (4) x tensor-parallel over heads (2x8).
Each core computes, for its (batch, 8-head group):
    qkv projection -> per-head causal attention (scoresT layout) -> partial out proj.
Host sums the two head-group partials per batch.

Layout choices (all matmuls in fp16):
 - x is pre-transposed on host: xT [DIM, T] so q/k land as [dh, t] ("T layouts")
   and v as [t, dh] with zero on-device transposes.
 - scores are computed transposed (scoresT[j, i] = k_j . q_i) so that
   exp(scoresT) can be used directly as the moving operand of the AV matmul and
   softmax sums come free via a ones-column appended to v.
 - rel-pos bias + causal mask are precomputed per head on host as a shifted
   multiplicative matrix Bh[p, c] = exp(bias(idx) | -inf), idx = p - c + 2431,
   so every scoresT tile's bias is a strided slice, applied post-exp on DVE.
 - the head pair (2g, 2g+1) lives at partition rows 0:64 / 64:128 of qT/kT, so
   their dh=64-contraction scores matmuls occupy disjoint PE row groups and
   run concurrently when issued back-to-back.
 - scores psum tiles hold 2 j-blocks ([128, 2, 512] = 2 banks) so exp and the
   bias multiply run as single 1024-wide ACT/DVE ops.
 - softmax normalization is deferred: the AV accumulator drains to SBUF at
   once (fast PSUM recycle); the sums row then bounces through DRAM for a
   partition-broadcast, is reciprocal'd and multiplied in on DVE one
   head-pair block later.
 - stage 1 (qkv) is chunk-interleaved with attention: attention on i-chunk ic
   overlaps the qkv projection of chunk ic+1 and the out-projection of chunk
   ic-1, keeping PE busy while ACT does exps.
"""

import os

import numpy as np

import concourse.bass as bass
import concourse.tile as tile
from concourse import bacc, mybir
from concourse.bass_utils import run_bass_kernel_spmd

B, T, DIM, HEADS, DH = 4, 2048, 1024, 16, 64
N_CORES = 8
HPC = HEADS // 2          # heads per core = 8
FQ = HPC * DH             # per-core q/k/v feature width = 512
BH_C = 2432               # bias matrix free size  (max shift 1920 + 512)

F32 = mybir.dt.float32
F16 = mybir.dt.float16
DTM = F16                 # matmul-input dtype
EXP = mybir.ActivationFunctionType.Exp
COPY = mybir.ActivationFunctionType.Copy
MULT = mybir.AluOpType.mult

_CACHE = {}


def build_nc():
    nc = bacc.Bacc("TRN2", target_bir_lowering=False, debug=False,
                   enable_asserts=True, num_devices=N_CORES)
    xT_d = nc.dram_tensor("xT", [DIM, T], DTM, kind="ExternalInput").ap()
    wq_d = nc.dram_tensor("wq", [DIM, FQ], DTM, kind="ExternalInput").ap()
    wk_d = nc.dram_tensor("wk", [DIM, FQ], DTM, kind="ExternalInput").ap()
    wv_d = nc.dram_tensor("wv", [DIM, FQ], DTM, kind="ExternalInput").ap()
    w0_d = nc.dram_tensor("w0", [FQ, DIM], DTM, kind="ExternalInput").ap()
    bh_d = nc.dram_tensor("bh", [HPC, 128, BH_C], DTM, kind="ExternalInput").ap()
    out_d = nc.dram_tensor("out", [T, DIM], F32, kind="ExternalOutput").ap()
    kdebug = os.environ.get("KDEBUG")
    if kdebug:
        ao_dbg = nc.dram_tensor("ao_dbg", [128, 4, T], DTM,
                                kind="ExternalOutput").ap()
        e_dbg = nc.dram_tensor("e_dbg", [128, 2, 512], DTM,
                               kind="ExternalOutput").ap()
        rb_dbg = nc.dram_tensor("rb_dbg", [64, 512], F32,
                                kind="ExternalOutput").ap()
        sc_dbg = nc.dram_tensor("sc_dbg", [128, 2, 512], DTM,
                                kind="ExternalOutput").ap()

    krepeat = int(os.environ.get("KREPEAT", "1"))
    ablate = os.environ.get("KABLATE", "")
    with tile.TileContext(nc) as tc:
      import contextlib
      loop_cm = (tc.For_i(0, krepeat, 1, staggered_reset=True,
                            hint_engines=(mybir.EngineType.PE, mybir.EngineType.Activation, mybir.EngineType.DVE))
                 if krepeat > 1 else contextlib.nullcontext())
      with tc.tile_pool(name="cst", bufs=1) as cst, \
           tc.tile_pool(name="persist", bufs=1) as persist:
        # ---- persistent SBUF state (outside the bench loop) ----
        qT = persist.tile([128, 4, T], DTM)       # q^T: [(h%2)*64+d, g, t]
        kT = persist.tile([128, 4, T], DTM)
        v_sb = persist.tile([128, 16, HPC, DH + 1], DTM)  # [t%128, t//128, h, d|1]
        aoT = persist.tile([128, 4, T], DTM)      # attn-out^T rows (h%2)*64+d
        wq_sb = persist.tile([128, 8, FQ], DTM)
        wk_sb = persist.tile([128, 8, FQ], DTM)
        wv_sb = persist.tile([128, 8, FQ], DTM)
        w0_sb = persist.tile([128, 4, DIM], DTM)
        bh_sb = persist.tile([128, HPC, BH_C], DTM)

        def load_invariants():
            """Weight/bias loads. In bench (KREPEAT) mode these are emitted
            once as a prologue and again at each body END, so the next
            iteration's weights prefetch behind the current tail instead of
            gating the first matmuls after the back-edge."""
            for kd in range(8):
                nc.scalar.dma_start(wk_sb[:, kd, :], wk_d[kd * 128:(kd + 1) * 128, :])
            for kd in range(8):
                nc.scalar.dma_start(wq_sb[:, kd, :], wq_d[kd * 128:(kd + 1) * 128, :])
            for kd in range(8):
                nc.scalar.dma_start(wv_sb[:, kd, :], wv_d[kd * 128:(kd + 1) * 128, :])
            for kf in range(4):
                nc.sync.dma_start(w0_sb[:, kf, :], w0_d[kf * 128:(kf + 1) * 128, :])
            for h in range(HPC):
                nc.scalar.dma_start(bh_sb[:, h, :], bh_d[h])

        with loop_cm:
            _es = contextlib.ExitStack()
            psS = _es.enter_context(tc.tile_pool(name="psS", bufs=3, space="PSUM"))
            psP = _es.enter_context(tc.tile_pool(name="psP", bufs=1, space="PSUM"))
            ps1 = _es.enter_context(tc.tile_pool(name="ps1", bufs=1, space="PSUM"))
            xpool = _es.enter_context(tc.tile_pool(name="xpool", bufs=16))
            ep = _es.enter_context(tc.tile_pool(name="ep", bufs=8))
            npool = _es.enter_context(tc.tile_pool(name="npool", bufs=4))
            dramp = _es.enter_context(tc.tile_pool(name="dramp", bufs=4, space="DRAM"))
            stgp = _es.enter_context(tc.tile_pool(name="stgp", bufs=3))
            osp = _es.enter_context(tc.tile_pool(name="osp", bufs=4))

            load_invariants()
            nc.vector.memset(v_sb[:, :, :, DH], 1.0)
            # prewarm the ACT exp table set (~2.7us) while stage 1 runs on PE
            warm = cst.tile([1, 2], F32)
            nc.vector.memset(warm, 0.0)
            nc.scalar.activation(warm, warm, EXP)

            def load_x_chunk(tci):
                t0 = tci * 512
                xts = []
                for kd in range(8):
                    xt1 = xpool.tile([128, 512], DTM, name=f"xt{tci}_{kd}",
                                     tag="xt")
                    nc.sync.dma_start(
                        xt1, xT_d[kd * 128:(kd + 1) * 128, t0:t0 + 512])
                    xts.append(xt1)
                return xts

            def qkv_piece(tci, xts, piece, wide=False):
                """One of 12 psum-tile units for chunk tci.
                piece 0-7: q/k (mb), piece 8-11: v (tt). During chunk 0 the
                scores pool is idle, so `wide` borrows its 2-bank slots for
                extra buffering."""
                t0 = tci * 512
                if wide:
                    ps = psS.tile([128, 2, 512], F32, name="sc", tag="sc")[:, 0, :]
                else:
                    ps = ps1.tile([128, 512], F32, name="ps1", tag="mm1")
                if piece < 8:
                    mb = piece
                    w_sb, fb = (wq_sb, mb) if mb < 4 else (wk_sb, mb - 4)
                    for kd in range(8):
                        nc.tensor.matmul(
                            ps, w_sb[:, kd, fb * 128:(fb + 1) * 128],
                            xts[kd], start=(kd == 0), stop=(kd == 7))
                    dst = qT if mb < 4 else kT
                    nc.vector.tensor_copy(dst[:, fb, t0:t0 + 512], ps)
                else:
                    tt = piece - 8
                    for kd in range(8):
                        nc.tensor.matmul(
                            ps, xts[kd][:, tt * 128:(tt + 1) * 128],
                            wv_sb[:, kd, :], start=(kd == 0), stop=(kd == 7))
                    tb = tci * 4 + tt
                    nc.scalar.activation(
                        v_sb[:, tb, :, 0:DH],
                        ps.rearrange("p (h d) -> p h d", h=HPC), COPY)

            def attn_front(g, ic, n):
                """SC + exp + bias-mult for pair-group n of (g, ic); returns
                the two heads' e tiles. Scores of the two heads issue
                adjacently so their row-disjoint 64-contraction matmuls
                overlap on the PE. The last group of each (g, ic) is >=half
                causally masked (its j-blocks start 256/384 columns into the
                i-window), so all its work runs on the 256-column suffix."""
                i0 = ic * 512
                jb0, jb1 = 2 * n, 2 * n + 1
                tr = 256 if n == 2 * (ic + 1) - 1 else 0
                # psum layout dim1: k=0 <-> jb1, k=1 <-> jb0 (so the bias
                # window AP walks +128 columns from the jb1 offset)
                pss = []
                for idx in range(2):
                    hp = idx * 64
                    ps = psS.tile([128, 2, 512], F32, name="sc", tag="sc")
                    for k, jb in ((0, jb1), (1, jb0)):
                        nc.tensor.matmul(
                            ps[:, k, tr:512],
                            kT[hp:hp + 64, g, jb * 128:jb * 128 + 128],
                            qT[hp:hp + 64, g, i0 + tr:i0 + 512],
                            start=True, stop=True)
                    pss.append(ps)
                es = []
                for idx in range(2):
                    h = 2 * g + idx
                    e = ep.tile([128, 2, 512], DTM, name="e", tag="e")
                    if ablate == "lean":
                        nc.vector.tensor_copy(e, pss[idx])
                        es.append(e)
                        continue
                    nc.scalar.activation(e[:, :, tr:512], pss[idx][:, :, tr:512],
                                         EXP)
                    if kdebug and (g, ic, n, idx) == (0, 0, 0, 0):
                        nc.sync.dma_start(sc_dbg, e)
                    off1 = i0 - jb1 * 128 + 384
                    w = bh_sb[:, h, off1 + tr:off1 + 512]
                    bwin = bass.AP(
                        tensor=w.tensor, offset=w.offset,
                        ap=[list(w.ap[0]), [128, 2], list(w.ap[1])])
                    nc.vector.tensor_tensor(e[:, :, tr:512], e[:, :, tr:512],
                                            bwin, MULT)
                    if kdebug and (g, ic, n, idx) == (0, 0, 0, 0):
                        nc.sync.dma_start(e_dbg, e)
                    es.append(e)
                return es

            def attn_av(g, ic, n, nlast, es, pos_pair):
                """AV accumulation for pair-group n — issued with a software
                lag behind its SC/exp/mult so the in-order PE stream never
                waits on a fresh exp+mult chain."""
                jb0, jb1 = 2 * n, 2 * n + 1
                tr = 256 if n == nlast else 0
                for idx in range(2):
                    h = 2 * g + idx
                    for k, jb in ((1, jb0), (0, jb1)):
                        nc.tensor.matmul(
                            pos_pair[idx][:, tr:512], v_sb[:, jb, h, :],
                            es[idx][:, k, tr:512],
                            start=(jb == 0), stop=(jb == 2 * nlast + 1),
                            skip_group_check=True)

            def pos_drain(pos_pair):
                """Copy the pair's AV accumulators to SBUF so the PSUM banks
                recycle fast; the softmax division happens later, off the
                accumulator-recycle loop."""
                outs = []
                for idx in range(2):
                    sa = npool.tile([DH + 1, 512], F32, name="sa", tag="sa",
                                    bufs=6)
                    nc.vector.tensor_copy(sa, pos_pair[idx])
                    outs.append(sa)
                return outs

            def normalize(g, ic, sas):
                i0 = ic * 512
                for idx in range(2):
                    sa = sas[idx]
                    if ablate == "lean":
                        if idx == 0:
                            nc.vector.tensor_copy(
                                aoT[0:64, g, i0:i0 + 512], sa[0:DH, :])
                        else:
                            stg = stgp.tile([64, 512], DTM, name="stg",
                                            tag="stg")
                            nc.vector.tensor_copy(stg, sa[0:DH, :])
                            nc.gpsimd.dma_start(
                                out=aoT[64:128, g, i0:i0 + 512], in_=stg)
                        continue
                    r_dram = dramp.tile([1, 512], F32, name="rd")
                    # scalar HWDGE: idle after the weight/bias prologue, so
                    # the latency-critical bounce isn't stuck behind x-chunk
                    # loads and output stores on the sync ring
                    nc.scalar.dma_start(r_dram, sa[DH:DH + 1, :])
                    rb = npool.tile([64, 512], F32, name="rb", tag="rb")
                    rb_src = bass.AP(tensor=r_dram.tensor, offset=r_dram.offset,
                                     ap=[[0, 64]] + list(r_dram.ap[1:]))
                    nc.gpsimd.dma_start(out=rb, in_=rb_src)
                    rr = npool.tile([64, 512], F32, name="rr", tag="rr")
                    nc.vector.reciprocal(rr, rb)
                    if kdebug and (g, ic, idx) == (0, 0, 0):
                        nc.sync.dma_start(rb_dbg, rr)
                    if idx == 0:
                        nc.vector.tensor_tensor(
                            aoT[0:64, g, i0:i0 + 512], sa[0:DH, :], rr, MULT)
                    else:
                        stg = stgp.tile([64, 512], DTM, name="stg", tag="stg")
                        nc.vector.tensor_tensor(stg, sa[0:DH, :], rr, MULT)
                        nc.gpsimd.dma_start(
                            out=aoT[64:128, g, i0:i0 + 512], in_=stg)

            def stage3_piece(tb, ncol, wide=False):
                n0 = ncol * 512
                if wide:
                    ps = psS.tile([128, 2, 512], F32, name="sc",
                                  tag="sc")[:, 0, :]
                else:
                    ps = ps1.tile([128, 512], F32, name="ps1", tag="mm1")
                for kf in range(4):
                    nc.tensor.matmul(
                        ps, aoT[:, kf, tb * 128:(tb + 1) * 128],
                        w0_sb[:, kf, n0:n0 + 512],
                        start=(kf == 0), stop=(kf == 3))
                o_sb = osp.tile([128, 512], F32, name="osb", tag="osb")
                if (tb + ncol) % 2 == 0:
                    nc.scalar.activation(o_sb, ps, COPY)
                    nc.sync.dma_start(
                        out_d[tb * 128:(tb + 1) * 128, n0:n0 + 512], o_sb)
                else:
                    nc.vector.tensor_copy(o_sb, ps)
                    nc.scalar.dma_start(
                        out_d[tb * 128:(tb + 1) * 128, n0:n0 + 512], o_sb)

            # ---- pipeline ----
            if ablate == "dma":
                # loads + output stores only: measures the DMA floor
                for tci in range(4):
                    load_x_chunk(tci)
                o_sb = osp.tile([128, 512], F32, name="osb", tag="osb")
                nc.vector.memset(o_sb, 0.0)
                for tb in range(16):
                    for ncol in range(2):
                        nc.sync.dma_start(
                            out_d[tb * 128:(tb + 1) * 128,
                                  ncol * 512:ncol * 512 + 512], o_sb)
            elif ablate == "noattn":
                # stage 1 + stage 3 only (aoT = zeros)
                nc.vector.memset(aoT, 0.0)
                for tci in range(4):
                    xts = load_x_chunk(tci)
                    for piece in range(12):
                        qkv_piece(tci, xts, piece)
                for tb in range(16):
                    for ncol in range(2):
                        stage3_piece(tb, ncol)
            elif ablate in ("dve2d", "dve3d"):
                # DVE micro-bench: 400 fp16 [128,1024] multiplies,
                # contiguous (2d) or bias-window-strided in2 (3d)
                ea = ep.tile([128, 2, 512], DTM, name="e", tag="e")
                nc.vector.memset(ea, 1.0)
                nc.vector.memset(bh_sb[:, 0, :], 1.0)
                for it in range(400):
                    off1 = (it * 128) % 1280 + 128
                    if ablate == "dve2d":
                        bw = bh_sb[:, 0, off1:off1 + 1024]
                        bw = bass.AP(tensor=bw.tensor, offset=bw.offset,
                                     ap=[list(bw.ap[0]), [512, 2], [1, 512]])
                    else:
                        w = bh_sb[:, 0, off1:off1 + 512]
                        bw = bass.AP(tensor=w.tensor, offset=w.offset,
                                     ap=[list(w.ap[0]), [128, 2], [1, 512]])
                    nc.vector.tensor_tensor(ea, ea, bw, MULT)
                nc.sync.dma_start(out_d[0:128, 0:512].bitcast(DTM),
                                  ea.rearrange("p a b -> p (a b)"))
            elif ablate == "attnonly":
                # attention only on memset activations (no stage 1/3)
                nc.vector.memset(qT, 0.01)
                nc.vector.memset(kT, 0.01)
                nc.vector.memset(v_sb[:, :, :, 0:DH], 0.01)
                deferred = []
                for ic in range(4):
                    ngrp = 2 * (ic + 1)
                    for g in range(4):
                        pos_pair = [
                            psP.tile([DH + 1, 512], F32, name=f"po{idx}",
                                     tag="po")
                            for idx in range(2)]
                        pend = []
                        for n in range(ngrp):
                            pend.append((n, attn_front(g, ic, n)))
                            if len(pend) > 2:
                                m, es = pend.pop(0)
                                attn_av(g, ic, m, ngrp - 1, es, pos_pair)
                            if deferred:
                                normalize(*deferred.pop(0))
                        for m, es in pend:
                            attn_av(g, ic, m, ngrp - 1, es, pos_pair)
                        deferred.append((g, ic, pos_drain(pos_pair)))
                for item in deferred:
                    normalize(*item)
                nc.sync.dma_start(out_d[0:128, 0:512].bitcast(DTM),
                                  aoT[:, 0, 0:1024])
            else:
                # chunk 0: k/q(g=0)/v first so attention(0) can start earliest;
                # alternate between the idle scores-pool slots and ps1.
                xts = load_x_chunk(0)
                for i, piece in enumerate([4, 5, 6, 7, 0, 8, 9, 10, 11, 1, 2, 3]):
                    qkv_piece(0, xts, piece, wide=(i % 3 != 2))

                stage3_todo = []
                deferred = []
                for ic in range(4):
                    if ic < 3:
                        xts_next = load_x_chunk(ic + 1)
                    ngrp = 2 * (ic + 1)
                    piece = 0
                    for g in range(4):
                        pos_pair = [
                            psP.tile([DH + 1, 512], F32, name=f"po{idx}",
                                     tag="po")
                            for idx in range(2)]
                        pend = []
                        for n in range(ngrp):
                            pend.append((n, attn_front(g, ic, n)))
                            if len(pend) > 2:
                                m, es = pend.pop(0)
                                attn_av(g, ic, m, ngrp - 1, es, pos_pair)
                            # interleave stage-1 pieces of the next chunk
                            if ic < 3 and n % 2 == 1 and piece < 12:
                                qkv_piece(ic + 1, xts_next, piece)
                                piece += 1
                            # interleave deferred out-projection tiles
                            elif stage3_todo and n % 2 == 1:
                                stage3_piece(*stage3_todo.pop(0))
                            # finish a deferred softmax division last so its
                            # bounce DMA has maximal slack before the DVE
                            # stream reaches the dependent multiply
                            if deferred:
                                normalize(*deferred.pop(0))
                        for m, es in pend:
                            attn_av(g, ic, m, ngrp - 1, es, pos_pair)
                        deferred.append((g, ic, pos_drain(pos_pair)))
                    if ic < 3:
                        while piece < 12:
                            qkv_piece(ic + 1, xts_next, piece)
                            piece += 1
                    while deferred:
                        normalize(*deferred.pop(0))
                    for tb in range(4 * ic, 4 * ic + 4):
                        for ncol in range(2):
                            stage3_todo.append((tb, ncol))
                for i, (tb, ncol) in enumerate(stage3_todo):
                    stage3_piece(tb, ncol, wide=(i % 3 != 2))
                if kdebug:
                    nc.sync.dma_start(ao_dbg, aoT)
            _es.close()
    nc.compile()
    return nc


def conv(a):
    return np.ascontiguousarray(a, dtype=np.float32).astype(np.float16)


def prep_inputs(x, W_qkv, W_0, rel_bias):
    """Shard + lay out the full inputs into 8 per-core input maps."""
    x = np.asarray(x, dtype=np.float32)
    W_qkv = np.asarray(W_qkv, dtype=np.float32)
    W_0 = np.asarray(W_0, dtype=np.float32)
    rel_bias = np.asarray(rel_bias, dtype=np.float32)

    # W_qkv columns are laid out (d, s, h): col = d*48 + s*16 + h
    wslab = W_qkv.reshape(DIM, DH, 3, HEADS)

    # bias matrices: bh_all[h, p, c] = exp(bias) | 0 at idx = p - c + 2431
    p = np.arange(128)[:, None]
    c = np.arange(BH_C)[None, :]
    idx = p - c + 2431                       # [128, C]
    safe = np.clip(idx, 0, 2 * T - 2)
    base = rel_bias[safe, :]                 # [128, C, HEADS]
    invalid = (idx < 0) | (idx > 2 * T - 2)
    masked = idx > T - 1                     # j > i  -> causal mask
    bh_all = np.where(masked[..., None], np.float32(-60000.0),
                      np.where(invalid[..., None], np.float32(0.0), base))
    bh_all = np.transpose(bh_all, (2, 0, 1)).copy()  # [HEADS, 128, C]
    bh_all = np.exp(bh_all)  # multiplicative form; exp(-60000) -> 0
    bh_all = conv(bh_all)

    in_maps = []
    for core in range(N_CORES):
        b, hg = divmod(core, 2)
        h0 = hg * HPC
        # per-core weight slices, feature order f = h*64 + d
        wq = wslab[:, :, 0, h0:h0 + HPC].transpose(0, 2, 1).reshape(DIM, FQ)
        wq = wq * np.float32(DH ** -0.5)
        wk = wslab[:, :, 1, h0:h0 + HPC].transpose(0, 2, 1).reshape(DIM, FQ)
        wv = wslab[:, :, 2, h0:h0 + HPC].transpose(0, 2, 1).reshape(DIM, FQ)
        in_maps.append({
            "xT": conv(x[b].T),
            "wq": conv(wq),
            "wk": conv(wk),
            "wv": conv(wv),
            "w0": conv(W_0[h0 * DH:(h0 + HPC) * DH, :]),
            "bh": np.ascontiguousarray(bh_all[h0:h0 + HPC]),
        })
    return in_maps


def kernel(x, W_qkv, W_0, rel_bias):
    if "nc" not in _CACHE:
        _CACHE["nc"] = build_nc()
    nc = _CACHE["nc"]
    in_maps = prep_inputs(x, W_qkv, W_0, rel_bias)
    res = run_bass_kernel_spmd(nc, in_maps, core_ids=list(range(N_CORES)))
    out = np.empty((B, T, DIM), dtype=np.float32)
    for b in range(B):
        out[b] = res.results[2 * b]["out"] + res.results[2 * b + 1]["out"]
    return out
